# revision 15
# baseline (speedup 1.0000x reference)
"""ChebNetConv (K=4) distributed Bass kernel for 8 Trainium2 NeuronCores.

Strategy (graph/data parallel, pull-mode SpMM):
  - Nodes are permuted into 8x6656 padded "slots" by a degree-balanced packer.
    Core c owns output slots [6656c, 6656(c+1)). Each window of 32 slots
    receives <=256 edges from even-parity source slots and <=256 from odd
    (2+2 chunks of 128 edge-positions).
  - SpMM: per chunk, psum[96f, 32r] += G[128e, 96f].T @ S[128e, 32r] on the
    TensorEngine, where G = dma_gather'ed source rows (f16, 256B rows) and
    S holds the Laplacian values (one column per dest row in the window).
  - Gather sources are parity-split tensors (<=26624 rows each, int16-indexable)
    storing 2*T_k in f16; AllGather (per parity) shares T_k across cores.
  - Chebyshev recurrence T_k = (2 L T_{k-1}) - T_{k-2} computed in the
    per-group epilogue; final out.T = sum_k W_k_fm.T @ T_k_fm + b on-chip.
"""
import numpy as np

import concourse.bass as bass
import concourse.bacc as bacc
import concourse.mybir as mybir
import concourse.tile as tile
from concourse.bass_utils import run_bass_kernel_spmd

f16 = np.float16

N_CORES = 8
ROWS_PER_CORE = 6656
WIN_ROWS = 32
HALF_CAP = 256
WINS_PER_CORE = ROWS_PER_CORE // WIN_ROWS       # 208
GROUPS_PER_CORE = ROWS_PER_CORE // 128          # 52
GPAIRS = GROUPS_PER_CORE // 2                   # 26
CHUNKS_PER_CORE = WINS_PER_CORE * 4             # 832
NPAD = N_CORES * ROWS_PER_CORE                  # 53248
HALF_ROWS = NPAD // 2                           # 26624
LOC_HALF = ROWS_PER_CORE // 2                   # 3328
IN_F, OUT_F, K = 96, 128, 4
EF = 128                                        # padded row elements (f16, 256B)
NIDX = 2048                                     # indices per dma_gather call

_compiled = None


# --------------------------------------------------------------------------
# host-side packing
# --------------------------------------------------------------------------

def _pack_rows(lap_rows, lap_cols, n):
    last = None
    for seed in range(8):
        try:
            return _pack_rows_seed(lap_rows, lap_cols, n, seed)
        except RuntimeError as e:
            last = e
    raise last


def _pack_rows_seed(lap_rows, lap_cols, n, seed=0):
    rng = np.random.default_rng(seed)
    label = np.zeros(n, np.int8)
    label[rng.permutation(n)[n // 2:]] = 1
    deg_a = np.bincount(lap_rows[label[lap_cols] == 0], minlength=n).astype(np.int64)
    deg_b = np.bincount(lap_rows[label[lap_cols] == 1], minlength=n).astype(np.int64)
    order = np.argsort(-(deg_a + deg_b), kind="stable")
    n_wins = N_CORES * WINS_PER_CORE
    wa = np.zeros(n_wins, np.int64)
    wb = np.zeros(n_wins, np.int64)
    wre = np.zeros(n_wins, np.int64)
    wro = np.zeros(n_wins, np.int64)
    row_slot = np.full(n, -1, np.int64)
    HR = WIN_ROWS // 2
    for r in order:
        a, b = deg_a[r], deg_b[r]
        lab = label[r]
        wrp = wro if lab else wre
        feas = (wrp < HR) & (wa + a <= HALF_CAP) & (wb + b <= HALF_CAP)
        if not feas.any():
            raise RuntimeError("window packing failed; graph too skewed")
        load = np.maximum(np.maximum((wa + a) / HALF_CAP, (wb + b) / HALF_CAP),
                          np.maximum((wre + (1 - lab)) / HR, (wro + lab) / HR))
        load[~feas] = 1e9
        w = int(np.argmin(load))
        pos = wro[w] * 2 + 1 if lab else wre[w] * 2
        row_slot[r] = w * WIN_ROWS + pos
        if lab:
            wro[w] += 1
        else:
            wre[w] += 1
        wa[w] += a
        wb[w] += b
    return row_slot


def _build_chunks(row_slot, lap_rows, lap_cols, lap_vals):
    """cols_half [NCHUNKS,128] int16 (source index within parity half),
    S [NCHUNKS,128,32] f32. chunk_id = gwin*16 + half*8 + (w%4)*2 + cin."""
    e_slot = row_slot[lap_rows]
    e_src = row_slot[lap_cols]
    e_half = (e_src % 2).astype(np.int64)
    e_win = e_slot // WIN_ROWS
    e_wr = e_slot % WIN_ROWS
    order = np.lexsort((e_half, e_win))
    ew, eh, ewr, esrc, ev = (e_win[order], e_half[order], e_wr[order],
                             e_src[order], lap_vals[order])
    n_wins = N_CORES * WINS_PER_CORE
    key = ew * 2 + eh
    start = np.searchsorted(key, np.arange(n_wins * 2))
    pos = np.arange(len(ew)) - start[key]
    assert pos.max() < HALF_CAP
    chunk = (ew // 4) * 16 + eh * 8 + (ew % 4) * 2 + pos // 128
    pin = pos % 128
    nchunks = n_wins * 4
    cols_half = np.zeros((nchunks, 128), np.int16)
    S = np.zeros((nchunks, 128, WIN_ROWS), np.float32)
    cols_half[chunk, pin] = (esrc // 2).astype(np.int16)
    S[chunk, pin, ewr] = ev
    return cols_half, S


def _idx_tile_per_core(cols_half_core):
    """[128, GROUPS_PER_CORE*128] int16 gather-index tile for one core.
    Call order: gp -> [A-call | B-call]; within a call, flat index i covers
    chunk j=i//128 (j//8 selects group 2gp+j//8, j%8 the chunk) pos i%128;
    wrapped at [i%16 replicated, call*128 + i//16]."""
    out = np.zeros((128, GROUPS_PER_CORE * 128), np.int16)
    for gp in range(GPAIRS):
        for half in (0, 1):
            call = gp * 2 + half
            flat = np.empty(NIDX, np.int16)
            for j in range(16):
                g = 2 * gp + j // 8
                cid = g * 16 + half * 8 + (j % 8)
                flat[j * 128:(j + 1) * 128] = cols_half_core[cid]
            blk = flat.reshape(128, 16).T               # [16, 128]
            out[:, call * 128:(call + 1) * 128] = np.tile(blk, (8, 1))
    return out


# --------------------------------------------------------------------------
# device graph
# --------------------------------------------------------------------------

def _build_nc():
    md = mybir.dt
    nc = bacc.Bacc(None, num_devices=N_CORES, num_swdge_queues=4,
                   dynamic_dma_scratch_size=32768)

    xe = nc.declare_dram_parameter("xe", [HALF_ROWS, EF], md.float16, isOutput=False)
    xo = nc.declare_dram_parameter("xo", [HALF_ROWS, EF], md.float16, isOutput=False)
    x0fm = nc.declare_dram_parameter("x0fm", [IN_F, ROWS_PER_CORE], md.float16, isOutput=False)
    scoef = nc.declare_dram_parameter("scoef", [128, CHUNKS_PER_CORE * WIN_ROWS], md.float16, isOutput=False)
    idx = nc.declare_dram_parameter("idx", [128, GROUPS_PER_CORE * 128], md.int16, isOutput=False)
    wfm = nc.declare_dram_parameter("wfm", [IN_F, K * OUT_F], md.float16, isOutput=False)
    bvec = nc.declare_dram_parameter("bvec", [OUT_F, 1], md.float32, isOutput=False)
    ident = nc.declare_dram_parameter("ident", [IN_F, IN_F], md.float16, isOutput=False)
    out = nc.declare_dram_parameter("out", [OUT_F, ROWS_PER_CORE], md.float32, isOutput=True)

    rg = [list(range(N_CORES))]

    with tile.TileContext(nc) as tc:
        with (
            tc.tile_pool(name="const", bufs=1) as cp,
            tc.tile_pool(name="gpool", bufs=10) as gpool,
            tc.tile_pool(name="ep", bufs=3) as ep,
            tc.tile_pool(name="ps_spmm", bufs=4, space="PSUM") as ps_spmm,
            tc.tile_pool(name="ps_t", bufs=2, space="PSUM") as ps_t,
            tc.tile_pool(name="ps_o", bufs=2, space="PSUM") as ps_o,
            tc.tile_pool(name="dram", bufs=1, space="DRAM") as dp,
        ):
            scoef_sb = cp.tile([128, CHUNKS_PER_CORE * WIN_ROWS], md.float16)
            idx_sb = cp.tile([128, GROUPS_PER_CORE * 128], md.int16)
            t0fm = cp.tile([IN_F, ROWS_PER_CORE], md.float16)
            t1fm = cp.tile([IN_F, ROWS_PER_CORE], md.float16)
            t2fm = cp.tile([IN_F, ROWS_PER_CORE], md.float16)
            t3fm = cp.tile([IN_F, ROWS_PER_CORE], md.float16)
            wfm_sb = cp.tile([IN_F, K * OUT_F], md.float16)
            b_sb = cp.tile([OUT_F, 1], md.float32)
            ident_sb = cp.tile([IN_F, IN_F], md.float16)

            nc.sync.dma_start(out=idx_sb[:, 0:256], in_=idx[:, 0:256])
            nc.sync.dma_start(out=idx_sb[:, 256:], in_=idx[:, 256:])
            nc.scalar.dma_start(out=scoef_sb[:], in_=scoef[:])
            nc.sync.dma_start(out=t0fm[:], in_=x0fm[:])
            nc.sync.dma_start(out=wfm_sb[:], in_=wfm[:])
            nc.sync.dma_start(out=b_sb[:], in_=bvec[:])
            nc.sync.dma_start(out=ident_sb[:], in_=ident[:])

            tfm = [t0fm, t1fm, t2fm, t3fm]

            # internal DRAM for the T_k exchange (k = 1, 2)
            tloc = {}
            tfull = {}
            for k in (1, 2):
                for h, tag in ((0, "e"), (1, "o")):
                    tloc[(k, h)] = dp.tile([LOC_HALF, EF], md.float16,
                                           name=f"t{k}{tag}loc")
                    tfull[(k, h)] = dp.tile([HALF_ROWS, EF], md.float16,
                                            addr_space="Shared", name=f"t{k}{tag}full")

            qn = 0
            for k in (1, 2, 3):
                src = (xe[:], xo[:]) if k == 1 else (tfull[(k - 1, 0)][:], tfull[(k - 1, 1)][:])
                if k >= 2:
                    for h2 in (0, 1):
                        nc.gpsimd.collective_compute(
                            "AllGather", mybir.AluOpType.bypass,
                            replica_groups=rg,
                            ins=[tloc[(k - 1, h2)][:]],
                            outs=[tfull[(k - 1, h2)][:]],
                        )
                for gp in range(GPAIRS):
                    gbuf = []
                    for half in (0, 1):
                        g_sb = gpool.tile([128, 16, EF], md.float16, tag="g",
                                          name=f"g_{k}_{half}_{gp}")
                        call = gp * 2 + half
                        nc.gpsimd.dma_gather(
                            g_sb[:], src[half],
                            idx_sb[:, call * 128:(call + 1) * 128],
                            NIDX, NIDX, EF,
                            single_packet=False, queue_num=qn,
                        )
                        qn = (qn + 1) % 4
                        gbuf.append(g_sb)
                    for h in (0, 1):
                        g = 2 * gp + h
                        psum = ps_spmm.tile([IN_F, 128], md.float32, space="PSUM", tag="pspmm")
                        for w in range(4):
                            for half in (0, 1):
                                for c in range(2):
                                    cid = g * 16 + half * 8 + w * 2 + c
                                    slot = h * 8 + w * 2 + c
                                    nc.tensor.matmul(
                                        out=psum[0:IN_F, w * WIN_ROWS:(w + 1) * WIN_ROWS],
                                        lhsT=gbuf[half][:, slot, 0:IN_F],
                                        rhs=scoef_sb[:, cid * WIN_ROWS:(cid + 1) * WIN_ROWS],
                                        start=(half == 0 and c == 0),
                                        stop=(half == 1 and c == 1),
                                    )
                        gsl = slice(g * 128, (g + 1) * 128)
                        if k == 1:
                            nc.scalar.copy(out=tfm[1][:, gsl], in_=psum[:])
                        else:
                            nc.vector.tensor_sub(tfm[k][:, gsl], psum[:], tfm[k - 2][:, gsl])
                        if k < 3:
                            fm2 = ep.tile([IN_F, 128], md.float16, tag="fm2")
                            nc.scalar.mul(out=fm2[:], in_=tfm[k][:, gsl], mul=2.0)
                            fm2r = fm2[:].rearrange("p (s two) -> p two s", two=2)
                            if g % 4 == 0:
                                rm4 = [ep.tile([64, 4, IN_F], md.float16,
                                               tag="rm4e", name=f"rm4e_{k}_{g}"),
                                       ep.tile([64, 4, IN_F], md.float16,
                                               tag="rm4o", name=f"rm4o_{k}_{g}")]
                            for h2 in (0, 1):
                                pst = ps_t.tile([64, IN_F], md.float16, space="PSUM", tag="pst")
                                nc.tensor.transpose(
                                    out=pst[:], in_=fm2r[:, h2, :],
                                    identity=ident_sb[:],
                                )
                                nc.scalar.copy(out=rm4[h2][:, g % 4, :], in_=pst[:])
                                if g % 4 == 3:
                                    dst = tloc[(k, h2)][:].rearrange(
                                        "(Q q r) f -> Q r q f", q=4, r=64)
                                    nc.sync.dma_start(
                                        out=dst[g // 4, :, :, 0:IN_F],
                                        in_=rm4[h2][:],
                                    )

            # final linear: out[c, rows] = sum_k W_k_fm.T @ T_k_fm + b
            for g in range(GROUPS_PER_CORE):
                gsl = slice(g * 128, (g + 1) * 128)
                po = ps_o.tile([OUT_F, 128], md.float32, space="PSUM", tag="po")
                for kk in range(K):
                    nc.tensor.matmul(
                        out=po[:],
                        lhsT=wfm_sb[:, kk * OUT_F:(kk + 1) * OUT_F],
                        rhs=tfm[kk][:, gsl],
                        start=(kk == 0),
                        stop=(kk == K - 1),
                    )
                osb = ep.tile([OUT_F, 128], md.float32, tag="osb")
                nc.vector.tensor_add(osb[:], po[:], b_sb[:, 0:1].to_broadcast([OUT_F, 128]))
                nc.sync.dma_start(out=out[:, gsl], in_=osb[:])

    nc.finalize()
    return nc


# --------------------------------------------------------------------------
# entry point
# --------------------------------------------------------------------------

def kernel(x, lap_rows, lap_cols, lap_vals, W, b):
    global _compiled
    x = np.asarray(x, np.float32)
    lap_rows = np.asarray(lap_rows, np.int32)
    lap_cols = np.asarray(lap_cols, np.int32)
    lap_vals = np.asarray(lap_vals, np.float32)
    W = np.asarray(W, np.float32)
    b = np.asarray(b, np.float32)
    n = x.shape[0]

    row_slot = _pack_rows(lap_rows, lap_cols, n)
    cols_half, S = _build_chunks(row_slot, lap_rows, lap_cols, lap_vals)

    x_pad = np.zeros((NPAD, IN_F), np.float32)
    x_pad[row_slot] = x
    xef = np.zeros((HALF_ROWS, EF), f16)
    xof = np.zeros((HALF_ROWS, EF), f16)
    xef[:, 0:IN_F] = x_pad[0::2].astype(f16)
    xof[:, 0:IN_F] = x_pad[1::2].astype(f16)

    Wr = W.reshape(OUT_F, IN_F, K)
    wfm = np.ascontiguousarray(
        Wr.transpose(1, 2, 0).reshape(IN_F, K * OUT_F)).astype(f16)
    bvec = b.reshape(OUT_F, 1).astype(np.float32)
    ident = np.eye(IN_F, dtype=f16)

    in_maps = []
    for c in range(N_CORES):
        csl = slice(c * CHUNKS_PER_CORE, (c + 1) * CHUNKS_PER_CORE)
        S_c = S[csl].astype(f16)                    # [832, 128, 32]
        scoef_c = np.ascontiguousarray(
            S_c.transpose(1, 0, 2).reshape(128, CHUNKS_PER_CORE * WIN_ROWS))
        idx_c = _idx_tile_per_core(cols_half[csl])
        x0fm_c = np.ascontiguousarray(
            x_pad[c * ROWS_PER_CORE:(c + 1) * ROWS_PER_CORE].T).astype(f16)
        in_maps.append({
            "xe": xef, "xo": xof, "x0fm": x0fm_c, "scoef": scoef_c,
            "idx": idx_c, "wfm": wfm, "bvec": bvec, "ident": ident,
        })

    global _last_in_maps
    _last_in_maps = in_maps
    if _compiled is None:
        _compiled = _build_nc()
    res = run_bass_kernel_spmd(_compiled, in_maps, core_ids=list(range(N_CORES)))
    out_pad = np.concatenate(
        [res.results[c]["out"] for c in range(N_CORES)], axis=1).T  # [NPAD, 128]
    return np.ascontiguousarray(out_pad[row_slot]).astype(np.float32)


if __name__ == "__main__":
    import time
    d = np.load("inputs.npz")
    t0 = time.time()
    y = kernel(**{k: d[k] for k in d.files})
    print(f"kernel {time.time()-t0:.1f}s")
    expected = np.load("expected.npy")
    rel = np.linalg.norm(y - expected) / np.linalg.norm(expected)
    print(f"rel_err {rel:.3e}")
